# revision 36
# baseline (speedup 1.0000x reference)
"""Trainium2 Bass kernel for nn_EvolvableSNN (T=512, B=8, N=4096, LIF SNN).

Strategy
--------
The LIF dynamics with these parameters are sub-threshold: the membrane
potential equilibrium is ~tau_mem*tau_syn*cur ~= 1e-4 * cur, four orders of
magnitude below threshold=1.0, so no neuron ever spikes and the recurrent
feedback term is identically zero.  With zero feedback the scan is a LINEAR
time-invariant filter of the feedforward drive:

    ff    = input[:, :, :512] @ W_in                      # [T, B, N]
    mem_t = DT^2 * sum_{s<=t} g(t-s) * ff_s               # per (b, n)
    g(d)  = (b^(d+1) - a^(d+1)) / (b - a),  a = 1-DT/tau_syn, b = 1-DT/tau_mem
    spikes_t = (mem_t >= threshold)

so mem = GT.T @_time (x @ W_in) -- two chained dense matmuls, fully parallel
across (batch, neuron).  Validity is guarded by a rigorous norm bound
computed on the host:

    max|mem| <= DT^2 * sum_d g(d) * max_row||x_row||_2 * max_col||W_col||_2

(~2e-3 for the target inputs, vs threshold 1.0).  If the bound (inflated by
the mixed-precision error allowance, see below) does not clear
min(threshold) by a wide margin -- or the device reports any spike -- we
fall back to an exact sequential numpy port of the reference.  The first
spike of the no-feedback system coincides with the first spike of the true
system, so "no spikes under linearization" exactly implies correctness.

Numerics: stage 1 (time filter) runs in bf16 operands with fp32 PSUM
accumulation; stage 2 (x W_in product) runs in fp8-e4m3 DoubleRow (2x PE
throughput) with power-of-two scale factors sx (on xg, applied by the
Scalar-engine PSUM->SBUF copy) and sw (folded into W on the host).  The
threshold is pre-scaled by sx*sw on the host, so the comparison
(mem*sx*sw >= th*sx*sw) is exactly monotone-equivalent.  Spike values {0,1}
are exact in the bf16 output, which the host casts back to fp32.

Sharding: (NBG batch-groups x NNG neuron-column-groups) grid over 8 cores.
Each core runs the same program on its input slice; no collectives.
"""

import math

import numpy as np
import ml_dtypes

import concourse.bass as bass
import concourse.mybir as mybir
import concourse.tile as tile
from concourse import bacc, bass_utils

# Problem constants (hardcoded per harness contract).
T, B, N = 512, 8, 4096
IN = 512          # INPUT_SIZE
DT = 0.001
P = 128           # SBUF partitions
NCORES = 8

# Core grid: NBG batch-groups x NNG neuron-groups (NBG * NNG == NCORES).
NBG, NNG = 4, 2
NB_LOC = B // NBG          # batches per core
NW = N // NNG              # neuron columns per core
KI = IN // P               # contraction tiles over input dim (4)
KP = KI // 2               # DoubleRow contraction pair-tiles (2)
KT = T // P                # tiles over time dim (4)
NCH = NW // 512            # 512-wide n chunks per core
F32 = mybir.dt.float32
BF16 = mybir.dt.bfloat16
FP8 = mybir.dt.float8e4
NPBF16 = ml_dtypes.bfloat16
NPFP8 = ml_dtypes.float8_e4m3

MARGIN = 0.1               # abs margin to min(threshold) for the fast path

_compiled = {}             # cached compiled Bass modules, keyed by variant
LAST_RES = None            # last device results (for external profiling)


def _filter_taps(alpha: float, beta: float) -> np.ndarray:
    """g(d) * DT^2 for d = 0..T-1 (float64)."""
    d = np.arange(T, dtype=np.float64)
    if abs(beta - alpha) > 1e-12:
        g = (beta ** (d + 1) - alpha ** (d + 1)) / (beta - alpha)
    else:
        g = (d + 1) * alpha**d
    return g * DT * DT


def _build_gt(alpha: float, beta: float) -> np.ndarray:
    """GT[s, t] = DT^2 * g(t - s) for s <= t else 0 (upper-triangular)."""
    g = _filter_taps(alpha, beta)
    s = np.arange(T)
    diff = s[None, :] - s[:, None]  # diff[s, t] = t - s
    gt = np.where(diff >= 0, g[np.clip(diff, 0, T - 1)], 0.0)
    return gt.astype(np.float32)


def _build_device(uniform_th: bool):
    """Compile the per-core Tile kernel; returns the Bass module.

    Input layouts are pre-packed on the host so every DMA is one large
    fully-contiguous transfer:
      x  [NB_LOC*P, KT*IN]     row (b*P + p), col (k*IN + i) = x_b[k*P+p, i]
      w  [P, NCH, KP, 2, 512]  fp8, w[p, j, kp, i2, n]
                               = W_in[(2kp+i2)*128+p, j*512+n] * sw
      gt [P, KT*T]             row p, col (k*T + t) = GT[k*P+p, t]
      th [P, NW]               threshold * sx * sw, replicated rows
      sc [P, 3]                col 0: sx (stage-1 copy scale),
                               col 1: th[0]*sx*sw, col 2: -th[0]*sx*sw

    When uniform_th, a subset of threshold compares is offloaded from the
    (saturated) Vector engine: ScalarE copies the PSUM tile to SBUF bf16,
    then GpSimd does a 1-input tensor_scalar is_ge against sc[:, 1].
    """
    nc = bacc.Bacc(
        "TRN2", target_bir_lowering=False, debug=False, num_devices=NCORES
    )
    x = nc.dram_tensor("x", [NB_LOC * P, KT * IN], BF16, kind="ExternalInput").ap()
    w = nc.dram_tensor("w", [P, NCH, KP, 2, 512], FP8, kind="ExternalInput").ap()
    gt = nc.dram_tensor("gt", [P, KT * T], BF16, kind="ExternalInput").ap()
    th = (
        None
        if uniform_th
        else nc.dram_tensor("th", [P, NW], F32, kind="ExternalInput").ap()
    )
    sc = nc.dram_tensor("sc", [P, 3], F32, kind="ExternalInput").ap()
    spk = nc.dram_tensor("spk", [NB_LOC * T, NW], FP8, kind="ExternalOutput").ap()

    # which (mt, j2) compare chunks run on ScalarE as Sign+Relu pairs
    # (spike = relu(sign(mem - th)); differs from is_ge only at exact
    # equality, which the sub-threshold guard excludes) to relieve the
    # saturated Vector engine.  Uniform-threshold variant only: the bias
    # comes from sc[:, 1].
    offload = {(1, 1), (3, 0)} if uniform_th else set()

    with tile.TileContext(nc) as tc:
        with (
            tc.tile_pool(name="const", bufs=1) as cpool,
            tc.tile_pool(name="xin", bufs=2) as xpool,
            tc.tile_pool(name="xg", bufs=2) as xgpool,
            tc.tile_pool(name="sout", bufs=4) as spool,
            tc.tile_pool(name="mcp", bufs=3) as mpool,
            tc.tile_pool(name="ps1", bufs=2, space="PSUM") as ps1,
            tc.tile_pool(name="ps2", bufs=3, space="PSUM") as ps2,
        ):
            # spread input loads over independent DMA paths; stage-1
            # operands (sc -- it gates the PSUM-recycling copies -- then
            # gt, x0) first so PE starts ASAP
            sc_sb = cpool.tile([P, 3], F32, tag="sc")
            nc.sync.dma_start(sc_sb, sc)
            gt_sb = cpool.tile([P, KT * T], BF16, tag="gt")
            for k in range(KT):
                nc.sync.dma_start(
                    gt_sb[:, k * T : (k + 1) * T], gt[:, k * T : (k + 1) * T]
                )
            x_sb = {}
            x_sb[0] = xpool.tile([P, KT * IN], BF16, tag="x", name="x0")
            for k in range(KT):
                nc.scalar.dma_start(
                    x_sb[0][:, k * IN : (k + 1) * IN],
                    x[0:P, k * IN : (k + 1) * IN],
                )
            w_sb = cpool.tile([P, NCH, KP, 2, 512], FP8, tag="w")
            for j in range(NCH):
                nc.sync.dma_start(w_sb[:, j], w[:, j])
            th_sb = None
            if not uniform_th:
                th_sb = cpool.tile([P, NW], F32, tag="th")
                nc.scalar.dma_start(th_sb, th)

            for b in range(1, NB_LOC):
                x_sb[b] = xpool.tile([P, KT * IN], BF16, tag="x", name=f"x{b}")
                nc.gpsimd.dma_start(x_sb[b], x[b * P : (b + 1) * P, :])

            # stage 1 for ALL batches first so the PE never stalls on the
            # PSUM->SBUF copies of the batch it is about to consume
            xg_all = []
            for b in range(NB_LOC):
                # xgT[i, t] = sum_s x_b[s, i] * GT[s, t]  (bf16 operands)
                # GT[s, t] == 0 for t < s: s-tile k only feeds t >= 128k.
                xg_sb = [
                    xgpool.tile(
                        [P, 2, T], FP8, tag=f"xgp{kp}", name=f"xgp{b}_{kp}"
                    )
                    for kp in range(KP)
                ]
                for m in range(KI):
                    p1 = ps1.tile([P, T], F32, tag="p1")
                    for k in range(KT):
                        nc.tensor.matmul(
                            p1[:, k * P :],
                            x_sb[b][:, k * IN + m * P : k * IN + (m + 1) * P],
                            gt_sb[:, k * T + k * P : (k + 1) * T],
                            start=(k == 0),
                            stop=(k == KT - 1),
                            skip_group_check=True,
                        )
                    # PSUM->SBUF copy with runtime scale sx, cast to fp8
                    nc.scalar.activation(
                        xg_sb[m // 2][:, m % 2, :],
                        p1,
                        mybir.ActivationFunctionType.Copy,
                        scale=sc_sb[:, 0:1],
                    )
                xg_all.append(xg_sb)

            for b in range(NB_LOC):
                xg_sb = xg_all[b]
                # stage 2 (fp8 DoubleRow): mem[t, n] = sum_i xgT[i,t] W[i,n]
                # PSUM tiles are 1024 wide (2 matmul groups) so each
                # compare amortizes its fixed cost over 1024 columns.
                for mt in range(KT):
                    s_sb = spool.tile([P, NW], FP8, tag="s")
                    for j2 in range(NCH // 2):
                        p2 = ps2.tile([P, 1024], F32, tag="p2")
                        for jh in range(2):
                            j = 2 * j2 + jh
                            for kp in range(KP):
                                nc.tensor.matmul(
                                    p2[:, jh * 512 : (jh + 1) * 512],
                                    xg_sb[kp][:, :, mt * P : (mt + 1) * P],
                                    w_sb[:, j, kp],
                                    start=(kp == 0),
                                    stop=(kp == KP - 1),
                                    perf_mode=mybir.MatmulPerfMode.DoubleRow,
                                    skip_group_check=True,
                                )
                        s_out = s_sb[:, j2 * 1024 : (j2 + 1) * 1024]
                        if (mt, j2) in offload:
                            mcp = mpool.tile([P, 1024], F32, tag="mcp")
                            nc.scalar.activation(
                                mcp,
                                p2,
                                mybir.ActivationFunctionType.Sign,
                                bias=sc_sb[:, 2:3],
                            )
                            nc.scalar.activation(
                                s_out,
                                mcp,
                                mybir.ActivationFunctionType.Relu,
                            )
                        elif uniform_th:
                            nc.vector.tensor_scalar(
                                s_out,
                                p2,
                                sc_sb[:, 1:2],
                                None,
                                op0=mybir.AluOpType.is_ge,
                            )
                        else:
                            nc.vector.tensor_tensor(
                                s_out,
                                p2,
                                th_sb[:, j2 * 1024 : (j2 + 1) * 1024],
                                op=mybir.AluOpType.is_ge,
                            )
                    eng = nc.gpsimd if mt % 2 == 0 else nc.sync
                    eng.dma_start(
                        spk[b * T + mt * P : b * T + (mt + 1) * P, :], s_sb
                    )
    nc.compile()
    return nc


def _pow2_scale(target_max: float, value_max: float) -> float:
    """Largest power of two s with value_max * s <= target_max."""
    if value_max <= 0 or not np.isfinite(value_max):
        return 1.0
    return 2.0 ** math.floor(math.log2(target_max / value_max))


def _run_spmd_with_retry(nc, in_maps, trace=False, tries=3):
    """run_bass_kernel_spmd with retry: execution occasionally dies with a
    transient NRT error (device left wedged by a previous process).  A
    plain retry usually fails in-process, so later attempts reset the jax
    backend to get a fresh PJRT client."""
    import time as _time

    last = None
    for attempt in range(tries):
        try:
            return bass_utils.run_bass_kernel_spmd(
                nc, in_maps, core_ids=list(range(NCORES)), trace=trace
            )
        except Exception as e:  # noqa: BLE001
            last = e
            _time.sleep(2.0)
            try:
                import jax

                jax.clear_caches()
                jax.extend.backend.clear_backends()
            except Exception:  # noqa: BLE001
                pass
    raise last


def _run_device(x_bm, W_in, gt_np, threshold, sx, sw, trace=False):
    """Run the SPMD kernel; returns (spikes [T,B,N] f32, results obj)."""
    uniform_th = bool(np.all(threshold == threshold.flat[0]))
    if uniform_th not in _compiled:
        _compiled[uniform_th] = _build_device(uniform_th)
    nc = _compiled[uniform_th]
    x_bf = x_bm.astype(NPBF16)  # [B*T, IN]
    gt_bf = gt_np.astype(NPBF16)  # [T, T]
    gt_pack = np.ascontiguousarray(
        gt_bf.reshape(KT, P, T).transpose(1, 0, 2).reshape(P, KT * T)
    )
    x_pack_all = np.ascontiguousarray(
        x_bf.reshape(B, KT, P, IN).transpose(0, 2, 1, 3).reshape(B * P, KT * IN)
    )
    w_fp8 = (W_in.astype(np.float64) * sw).astype(np.float32).astype(NPFP8)
    th_scaled = (threshold.astype(np.float64) * (sx * sw)).astype(np.float32)
    sc_arr = np.empty((P, 3), dtype=np.float32)
    sc_arr[:, 0] = sx
    sc_arr[:, 1] = th_scaled.flat[0]
    sc_arr[:, 2] = -th_scaled.flat[0]
    in_maps = []
    for c in range(NCORES):
        bg, ng = divmod(c, NNG)
        xs = np.ascontiguousarray(
            x_pack_all[bg * NB_LOC * P : (bg + 1) * NB_LOC * P]
        )
        # w[p, j, kp, i2, n] = W_in[(2kp+i2)*128+p, ng*NW + j*512+n] * sw
        wc = np.ascontiguousarray(
            w_fp8[:, ng * NW : (ng + 1) * NW]
            .reshape(KP, 2, P, NCH, 512)
            .transpose(2, 3, 0, 1, 4)
        )
        thc = np.ascontiguousarray(
            np.broadcast_to(th_scaled[ng * NW : (ng + 1) * NW], (P, NW))
        )
        m = {"x": xs, "w": wc, "gt": gt_pack, "sc": sc_arr}
        if not uniform_th:
            m["th"] = thc
        in_maps.append(m)
    res = _run_spmd_with_retry(nc, in_maps, trace=trace)
    global LAST_RES
    LAST_RES = res
    out = np.zeros((B, T, N), dtype=np.float32)
    for c in range(NCORES):
        bg, ng = divmod(c, NNG)
        s = res.results[c]["spk"].astype(np.float32).reshape(NB_LOC, T, NW)
        out[bg * NB_LOC : (bg + 1) * NB_LOC, :, ng * NW : (ng + 1) * NW] = s
    return out.transpose(1, 0, 2), res


def _fallback(input_signal, weights, tau_mem, tau_syn, threshold):
    """Exact sequential port of the reference (numpy float32)."""
    x = np.asarray(input_signal, dtype=np.float32)
    w = np.asarray(weights, dtype=np.float32)
    W_in, W_rec = w[:IN], w[IN:]
    Tt, Bb, Nn = x.shape
    ff = np.einsum("tbi,in->tbn", x[:, :, :IN], W_in).astype(np.float32)
    syn = np.zeros((Bb, Nn), np.float32)
    mem = np.zeros((Bb, Nn), np.float32)
    fb = np.zeros((Bb, Nn), np.float32)
    out = np.zeros((Tt, Bb, Nn), np.float32)
    for t in range(Tt):
        cur = ff[t] + fb
        syn = syn + (-syn / tau_syn + cur) * np.float32(DT)
        mem = mem + (-mem / tau_mem + syn) * np.float32(DT)
        spikes = (mem >= threshold).astype(np.float32)
        mem = mem * (1.0 - spikes)
        rec = spikes[:, IN:] @ W_rec
        rec[:, :IN] = 0.0
        fb = rec
        out[t] = spikes
    return out


def kernel(input_signal, weights, tau_mem, tau_syn, threshold, _trace=False):
    input_signal = np.asarray(input_signal)
    weights = np.asarray(weights)
    tau_mem = np.asarray(tau_mem)
    tau_syn = np.asarray(tau_syn)
    threshold = np.asarray(threshold)

    ok_shape = (
        input_signal.shape == (T, B, N)
        and weights.shape == (N, N)
        and np.all(tau_mem == tau_mem.flat[0])
        and np.all(tau_syn == tau_syn.flat[0])
        and np.all(np.isfinite(input_signal))
        and np.all(np.isfinite(weights[:IN]))
        and np.all(np.isfinite(threshold))
    )
    if not ok_shape:
        return _fallback(input_signal, weights, tau_mem, tau_syn, threshold)

    alpha = 1.0 - DT / float(tau_syn.flat[0])
    beta = 1.0 - DT / float(tau_mem.flat[0])
    if not (0.0 <= alpha < 1.0 and 0.0 <= beta < 1.0):
        # numerically unstable / nonstandard regime: be safe
        return _fallback(input_signal, weights, tau_mem, tau_syn, threshold)

    gt_np = _build_gt(alpha, beta)

    # --- rigorous sub-threshold bound (exact arithmetic) -----------------
    # |mem[t,b,n]| <= max_i? no: mem = xg @ W with
    # |xg[i,t]| <= max_col||x_col||_2 * max_col||gt_col||_2
    # |mem[t,n]| <= ||xg[:,t]||_2 * ||W[:,n]||_2
    #            <= sum_d g(d)DT^2 * max_row||x_row||_2 * max_col||W_col||_2
    x_in = input_signal[:, :, :IN].astype(np.float64)
    W_in64 = weights[:IN].astype(np.float64)
    max_row = float(np.sqrt((x_in * x_in).sum(axis=2).max()))
    max_wcol = float(np.sqrt((W_in64 * W_in64).sum(axis=0).max()))
    gsum = float(_filter_taps(alpha, beta).sum())
    mem_bound = gsum * max_row * max_wcol

    # fp8 scale factors from data maxima / bounds (powers of two, exact)
    # xg bound: |xg[i,t]| <= max_i ||x[:,i]||_2 (per batch) * max_t ||gt[:,t]||_2
    xcol_max = float(
        np.sqrt(
            (x_in * x_in).sum(axis=0).max()  # sum over t for each (b, i)
        )
    )
    gtcol_max = float(np.sqrt((gt_np.astype(np.float64) ** 2).sum(axis=0).max()))
    xg_bound = xcol_max * gtcol_max
    w_max = float(np.abs(W_in64).max())
    sx = _pow2_scale(224.0, xg_bound)
    sw = _pow2_scale(224.0, w_max)

    # --- mixed-precision error allowance (conservative, absolute) -------
    # bf16 stage-1 relative error ~<1%; fp8 e4m3 operand rounding <=2^-4
    # relative each plus subnormal-flush floors eps = 2^-9/scale.
    eps_x = 2.0**-9 / sx
    eps_w = 2.0**-9 / sw
    err = (
        0.15 * mem_bound
        + IN * (eps_x * w_max + eps_w * xg_bound + eps_x * eps_w)
    )
    safe = (mem_bound + err) < float(threshold.min()) - MARGIN
    if not safe:
        return _fallback(input_signal, weights, tau_mem, tau_syn, threshold)

    # batch-major rows: row (b*T + t) = input_signal[t, b, :IN]
    x_bm = np.ascontiguousarray(
        input_signal[:, :, :IN].transpose(1, 0, 2).reshape(B * T, IN)
    ).astype(np.float32, copy=False)
    W_in = np.ascontiguousarray(weights[:IN]).astype(np.float32, copy=False)

    try:
        spikes, _ = _run_device(
            x_bm, W_in, gt_np, threshold.astype(np.float32), sx, sw,
            trace=_trace,
        )
    except Exception:  # device unusable: still return a correct result
        return _fallback(input_signal, weights, tau_mem, tau_syn, threshold)
    if spikes.any():
        # bound said sub-threshold yet device saw spikes: distrust, recompute
        return _fallback(input_signal, weights, tau_mem, tau_syn, threshold)
    return spikes


# revision 37
# speedup vs baseline: 1.0471x; 1.0471x over previous
"""Trainium2 Bass kernel for nn_EvolvableSNN (T=512, B=8, N=4096, LIF SNN).

Strategy
--------
The LIF dynamics with these parameters are sub-threshold: the membrane
potential equilibrium is ~tau_mem*tau_syn*cur ~= 1e-4 * cur, four orders of
magnitude below threshold=1.0, so no neuron ever spikes and the recurrent
feedback term is identically zero.  With zero feedback the scan is a LINEAR
time-invariant filter of the feedforward drive:

    ff    = input[:, :, :512] @ W_in                      # [T, B, N]
    mem_t = DT^2 * sum_{s<=t} g(t-s) * ff_s               # per (b, n)
    g(d)  = (b^(d+1) - a^(d+1)) / (b - a),  a = 1-DT/tau_syn, b = 1-DT/tau_mem
    spikes_t = (mem_t >= threshold)

so mem = GT.T @_time (x @ W_in) -- two chained dense matmuls, fully parallel
across (batch, neuron).  Validity is guarded by a rigorous norm bound
computed on the host:

    max|mem| <= DT^2 * sum_d g(d) * max_row||x_row||_2 * max_col||W_col||_2

(~2e-3 for the target inputs, vs threshold 1.0).  If the bound (inflated by
the mixed-precision error allowance, see below) does not clear
min(threshold) by a wide margin -- or the device reports any spike -- we
fall back to an exact sequential numpy port of the reference.  The first
spike of the no-feedback system coincides with the first spike of the true
system, so "no spikes under linearization" exactly implies correctness.

Numerics: stage 1 (time filter) runs in bf16 operands with fp32 PSUM
accumulation; stage 2 (x W_in product) runs in fp8-e4m3 DoubleRow (2x PE
throughput) with power-of-two scale factors sx (on xg, applied by the
Scalar-engine PSUM->SBUF copy) and sw (folded into W on the host).  The
threshold is pre-scaled by sx*sw on the host, so the comparison
(mem*sx*sw >= th*sx*sw) is exactly monotone-equivalent.  Spike values {0,1}
are exact in the bf16 output, which the host casts back to fp32.

Sharding: (NBG batch-groups x NNG neuron-column-groups) grid over 8 cores.
Each core runs the same program on its input slice; no collectives.
"""

import math

import numpy as np
import ml_dtypes

import concourse.bass as bass
import concourse.mybir as mybir
import concourse.tile as tile
from concourse import bacc, bass_utils

# Problem constants (hardcoded per harness contract).
T, B, N = 512, 8, 4096
IN = 512          # INPUT_SIZE
DT = 0.001
P = 128           # SBUF partitions
NCORES = 8

# Core grid: NBG batch-groups x NNG neuron-groups (NBG * NNG == NCORES).
NBG, NNG = 4, 2
NB_LOC = B // NBG          # batches per core
NW = N // NNG              # neuron columns per core
KI = IN // P               # contraction tiles over input dim (4)
KP = KI // 2               # DoubleRow contraction pair-tiles (2)
KT = T // P                # tiles over time dim (4)
NCH = NW // 512            # 512-wide n chunks per core
F32 = mybir.dt.float32
BF16 = mybir.dt.bfloat16
FP8 = mybir.dt.float8e4
NPBF16 = ml_dtypes.bfloat16
NPFP8 = ml_dtypes.float8_e4m3

MARGIN = 0.1               # abs margin to min(threshold) for the fast path

_compiled = {}             # cached compiled Bass modules, keyed by variant
LAST_RES = None            # last device results (for external profiling)


def _filter_taps(alpha: float, beta: float) -> np.ndarray:
    """g(d) * DT^2 for d = 0..T-1 (float64)."""
    d = np.arange(T, dtype=np.float64)
    if abs(beta - alpha) > 1e-12:
        g = (beta ** (d + 1) - alpha ** (d + 1)) / (beta - alpha)
    else:
        g = (d + 1) * alpha**d
    return g * DT * DT


def _build_gt(alpha: float, beta: float) -> np.ndarray:
    """GT[s, t] = DT^2 * g(t - s) for s <= t else 0 (upper-triangular)."""
    g = _filter_taps(alpha, beta)
    s = np.arange(T)
    diff = s[None, :] - s[:, None]  # diff[s, t] = t - s
    gt = np.where(diff >= 0, g[np.clip(diff, 0, T - 1)], 0.0)
    return gt.astype(np.float32)


def _build_device(uniform_th: bool):
    """Compile the per-core Tile kernel; returns the Bass module.

    Input layouts are pre-packed on the host so every DMA is one large
    fully-contiguous transfer:
      x  [NB_LOC*P, KT*IN]     row (b*P + p), col (k*IN + i) = x_b[k*P+p, i]
      w  [P, NCH, KP, 2, 512]  fp8, w[p, j, kp, i2, n]
                               = W_in[(2kp+i2)*128+p, j*512+n] * sw
      gt [P, KT*T]             row p, col (k*T + t) = GT[k*P+p, t]
      th [P, NW]               threshold * sx * sw, replicated rows
      sc [P, 3]                col 0: sx (stage-1 copy scale),
                               col 1: th[0]*sx*sw, col 2: -th[0]*sx*sw

    When uniform_th, a subset of threshold compares is offloaded from the
    (saturated) Vector engine: ScalarE copies the PSUM tile to SBUF bf16,
    then GpSimd does a 1-input tensor_scalar is_ge against sc[:, 1].
    """
    nc = bacc.Bacc(
        "TRN2", target_bir_lowering=False, debug=False, num_devices=NCORES
    )
    x = nc.dram_tensor("x", [NB_LOC * P, KT * IN], BF16, kind="ExternalInput").ap()
    w = nc.dram_tensor("w", [P, NCH, KP, 2, 512], FP8, kind="ExternalInput").ap()
    gt = nc.dram_tensor("gt", [P, KT * T], BF16, kind="ExternalInput").ap()
    th = (
        None
        if uniform_th
        else nc.dram_tensor("th", [P, NW], F32, kind="ExternalInput").ap()
    )
    sc = nc.dram_tensor("sc", [P, 3], F32, kind="ExternalInput").ap()
    spk = nc.dram_tensor("spk", [NB_LOC * T, NW], FP8, kind="ExternalOutput").ap()

    # which (mt, j2) compare chunks run on ScalarE as Sign+Relu pairs
    # (spike = relu(sign(mem - th)); differs from is_ge only at exact
    # equality, which the sub-threshold guard excludes) to relieve the
    # saturated Vector engine.  Uniform-threshold variant only: the bias
    # comes from sc[:, 1].
    offload = (
        {(0, 1), (1, 0), (2, 1), (3, 0), (3, 1), (2, 0)}
        if uniform_th
        else set()
    )

    with tile.TileContext(nc) as tc:
        with (
            tc.tile_pool(name="const", bufs=1) as cpool,
            tc.tile_pool(name="xin", bufs=2) as xpool,
            tc.tile_pool(name="xg", bufs=2) as xgpool,
            tc.tile_pool(name="sout", bufs=4) as spool,
            tc.tile_pool(name="mcp", bufs=3) as mpool,
            tc.tile_pool(name="ps1", bufs=2, space="PSUM") as ps1,
            tc.tile_pool(name="ps2", bufs=3, space="PSUM") as ps2,
        ):
            # spread input loads over independent DMA paths; stage-1
            # operands (sc -- it gates the PSUM-recycling copies -- then
            # gt, x0) first so PE starts ASAP
            sc_sb = cpool.tile([P, 3], F32, tag="sc")
            nc.sync.dma_start(sc_sb, sc)
            gt_sb = cpool.tile([P, KT * T], BF16, tag="gt")
            for k in range(KT):
                nc.sync.dma_start(
                    gt_sb[:, k * T : (k + 1) * T], gt[:, k * T : (k + 1) * T]
                )
            x_sb = {}
            x_sb[0] = xpool.tile([P, KT * IN], BF16, tag="x", name="x0")
            for k in range(KT):
                nc.scalar.dma_start(
                    x_sb[0][:, k * IN : (k + 1) * IN],
                    x[0:P, k * IN : (k + 1) * IN],
                )
            w_sb = cpool.tile([P, NCH, KP, 2, 512], FP8, tag="w")
            for j in range(NCH):
                nc.sync.dma_start(w_sb[:, j], w[:, j])
            th_sb = None
            if not uniform_th:
                th_sb = cpool.tile([P, NW], F32, tag="th")
                nc.scalar.dma_start(th_sb, th)

            for b in range(1, NB_LOC):
                x_sb[b] = xpool.tile([P, KT * IN], BF16, tag="x", name=f"x{b}")
                nc.gpsimd.dma_start(x_sb[b], x[b * P : (b + 1) * P, :])

            # stage 1 for ALL batches first so the PE never stalls on the
            # PSUM->SBUF copies of the batch it is about to consume
            xg_all = []
            for b in range(NB_LOC):
                # xgT[i, t] = sum_s x_b[s, i] * GT[s, t]  (bf16 operands)
                # GT[s, t] == 0 for t < s: s-tile k only feeds t >= 128k.
                xg_sb = [
                    xgpool.tile(
                        [P, 2, T], FP8, tag=f"xgp{kp}", name=f"xgp{b}_{kp}"
                    )
                    for kp in range(KP)
                ]
                for m in range(KI):
                    p1 = ps1.tile([P, T], F32, tag="p1")
                    for k in range(KT):
                        nc.tensor.matmul(
                            p1[:, k * P :],
                            x_sb[b][:, k * IN + m * P : k * IN + (m + 1) * P],
                            gt_sb[:, k * T + k * P : (k + 1) * T],
                            start=(k == 0),
                            stop=(k == KT - 1),
                            skip_group_check=True,
                        )
                    # PSUM->SBUF copy with runtime scale sx, cast to fp8
                    nc.scalar.activation(
                        xg_sb[m // 2][:, m % 2, :],
                        p1,
                        mybir.ActivationFunctionType.Copy,
                        scale=sc_sb[:, 0:1],
                    )
                xg_all.append(xg_sb)

            for b in range(NB_LOC):
                xg_sb = xg_all[b]
                # stage 2 (fp8 DoubleRow): mem[t, n] = sum_i xgT[i,t] W[i,n]
                # PSUM tiles are 1024 wide (2 matmul groups) so each
                # compare amortizes its fixed cost over 1024 columns.
                for mt in range(KT):
                    s_sb = spool.tile([P, NW], FP8, tag="s")
                    for j2 in range(NCH // 2):
                        p2 = ps2.tile([P, 1024], F32, tag="p2")
                        for jh in range(2):
                            j = 2 * j2 + jh
                            for kp in range(KP):
                                nc.tensor.matmul(
                                    p2[:, jh * 512 : (jh + 1) * 512],
                                    xg_sb[kp][:, :, mt * P : (mt + 1) * P],
                                    w_sb[:, j, kp],
                                    start=(kp == 0),
                                    stop=(kp == KP - 1),
                                    perf_mode=mybir.MatmulPerfMode.DoubleRow,
                                    skip_group_check=True,
                                )
                        s_out = s_sb[:, j2 * 1024 : (j2 + 1) * 1024]
                        if (mt, j2) in offload:
                            # sign(mem - th) in {-1, 0, 1}; host maps >0 to
                            # spikes, so no second Relu pass is needed
                            nc.scalar.activation(
                                s_out,
                                p2,
                                mybir.ActivationFunctionType.Sign,
                                bias=sc_sb[:, 2:3],
                            )
                        elif uniform_th:
                            nc.vector.tensor_scalar(
                                s_out,
                                p2,
                                sc_sb[:, 1:2],
                                None,
                                op0=mybir.AluOpType.is_ge,
                            )
                        else:
                            nc.vector.tensor_tensor(
                                s_out,
                                p2,
                                th_sb[:, j2 * 1024 : (j2 + 1) * 1024],
                                op=mybir.AluOpType.is_ge,
                            )
                    eng = nc.gpsimd if mt % 2 == 0 else nc.sync
                    eng.dma_start(
                        spk[b * T + mt * P : b * T + (mt + 1) * P, :], s_sb
                    )
    nc.compile()
    return nc


def _pow2_scale(target_max: float, value_max: float) -> float:
    """Largest power of two s with value_max * s <= target_max."""
    if value_max <= 0 or not np.isfinite(value_max):
        return 1.0
    return 2.0 ** math.floor(math.log2(target_max / value_max))


def _run_spmd_with_retry(nc, in_maps, trace=False, tries=3):
    """run_bass_kernel_spmd with retry: execution occasionally dies with a
    transient NRT error (device left wedged by a previous process).  A
    plain retry usually fails in-process, so later attempts reset the jax
    backend to get a fresh PJRT client."""
    import time as _time

    last = None
    for attempt in range(tries):
        try:
            return bass_utils.run_bass_kernel_spmd(
                nc, in_maps, core_ids=list(range(NCORES)), trace=trace
            )
        except Exception as e:  # noqa: BLE001
            last = e
            _time.sleep(2.0)
            try:
                import jax

                jax.clear_caches()
                jax.extend.backend.clear_backends()
            except Exception:  # noqa: BLE001
                pass
    raise last


def _run_device(x_bm, W_in, gt_np, threshold, sx, sw, trace=False):
    """Run the SPMD kernel; returns (spikes [T,B,N] f32, results obj)."""
    uniform_th = bool(np.all(threshold == threshold.flat[0]))
    if uniform_th not in _compiled:
        _compiled[uniform_th] = _build_device(uniform_th)
    nc = _compiled[uniform_th]
    x_bf = x_bm.astype(NPBF16)  # [B*T, IN]
    gt_bf = gt_np.astype(NPBF16)  # [T, T]
    gt_pack = np.ascontiguousarray(
        gt_bf.reshape(KT, P, T).transpose(1, 0, 2).reshape(P, KT * T)
    )
    x_pack_all = np.ascontiguousarray(
        x_bf.reshape(B, KT, P, IN).transpose(0, 2, 1, 3).reshape(B * P, KT * IN)
    )
    w_fp8 = (W_in.astype(np.float64) * sw).astype(np.float32).astype(NPFP8)
    th_scaled = (threshold.astype(np.float64) * (sx * sw)).astype(np.float32)
    sc_arr = np.empty((P, 3), dtype=np.float32)
    sc_arr[:, 0] = sx
    sc_arr[:, 1] = th_scaled.flat[0]
    sc_arr[:, 2] = -th_scaled.flat[0]
    in_maps = []
    for c in range(NCORES):
        bg, ng = divmod(c, NNG)
        xs = np.ascontiguousarray(
            x_pack_all[bg * NB_LOC * P : (bg + 1) * NB_LOC * P]
        )
        # w[p, j, kp, i2, n] = W_in[(2kp+i2)*128+p, ng*NW + j*512+n] * sw
        wc = np.ascontiguousarray(
            w_fp8[:, ng * NW : (ng + 1) * NW]
            .reshape(KP, 2, P, NCH, 512)
            .transpose(2, 3, 0, 1, 4)
        )
        thc = np.ascontiguousarray(
            np.broadcast_to(th_scaled[ng * NW : (ng + 1) * NW], (P, NW))
        )
        m = {"x": xs, "w": wc, "gt": gt_pack, "sc": sc_arr}
        if not uniform_th:
            m["th"] = thc
        in_maps.append(m)
    res = _run_spmd_with_retry(nc, in_maps, trace=trace)
    global LAST_RES
    LAST_RES = res
    out = np.zeros((B, T, N), dtype=np.float32)
    for c in range(NCORES):
        bg, ng = divmod(c, NNG)
        s = (
            (res.results[c]["spk"].astype(np.float32) > 0)
            .astype(np.float32)
            .reshape(NB_LOC, T, NW)
        )
        out[bg * NB_LOC : (bg + 1) * NB_LOC, :, ng * NW : (ng + 1) * NW] = s
    return out.transpose(1, 0, 2), res


def _fallback(input_signal, weights, tau_mem, tau_syn, threshold):
    """Exact sequential port of the reference (numpy float32)."""
    x = np.asarray(input_signal, dtype=np.float32)
    w = np.asarray(weights, dtype=np.float32)
    W_in, W_rec = w[:IN], w[IN:]
    Tt, Bb, Nn = x.shape
    ff = np.einsum("tbi,in->tbn", x[:, :, :IN], W_in).astype(np.float32)
    syn = np.zeros((Bb, Nn), np.float32)
    mem = np.zeros((Bb, Nn), np.float32)
    fb = np.zeros((Bb, Nn), np.float32)
    out = np.zeros((Tt, Bb, Nn), np.float32)
    for t in range(Tt):
        cur = ff[t] + fb
        syn = syn + (-syn / tau_syn + cur) * np.float32(DT)
        mem = mem + (-mem / tau_mem + syn) * np.float32(DT)
        spikes = (mem >= threshold).astype(np.float32)
        mem = mem * (1.0 - spikes)
        rec = spikes[:, IN:] @ W_rec
        rec[:, :IN] = 0.0
        fb = rec
        out[t] = spikes
    return out


def kernel(input_signal, weights, tau_mem, tau_syn, threshold, _trace=False):
    input_signal = np.asarray(input_signal)
    weights = np.asarray(weights)
    tau_mem = np.asarray(tau_mem)
    tau_syn = np.asarray(tau_syn)
    threshold = np.asarray(threshold)

    ok_shape = (
        input_signal.shape == (T, B, N)
        and weights.shape == (N, N)
        and np.all(tau_mem == tau_mem.flat[0])
        and np.all(tau_syn == tau_syn.flat[0])
        and np.all(np.isfinite(input_signal))
        and np.all(np.isfinite(weights[:IN]))
        and np.all(np.isfinite(threshold))
    )
    if not ok_shape:
        return _fallback(input_signal, weights, tau_mem, tau_syn, threshold)

    alpha = 1.0 - DT / float(tau_syn.flat[0])
    beta = 1.0 - DT / float(tau_mem.flat[0])
    if not (0.0 <= alpha < 1.0 and 0.0 <= beta < 1.0):
        # numerically unstable / nonstandard regime: be safe
        return _fallback(input_signal, weights, tau_mem, tau_syn, threshold)

    gt_np = _build_gt(alpha, beta)

    # --- rigorous sub-threshold bound (exact arithmetic) -----------------
    # |mem[t,b,n]| <= max_i? no: mem = xg @ W with
    # |xg[i,t]| <= max_col||x_col||_2 * max_col||gt_col||_2
    # |mem[t,n]| <= ||xg[:,t]||_2 * ||W[:,n]||_2
    #            <= sum_d g(d)DT^2 * max_row||x_row||_2 * max_col||W_col||_2
    x_in = input_signal[:, :, :IN].astype(np.float64)
    W_in64 = weights[:IN].astype(np.float64)
    max_row = float(np.sqrt((x_in * x_in).sum(axis=2).max()))
    max_wcol = float(np.sqrt((W_in64 * W_in64).sum(axis=0).max()))
    gsum = float(_filter_taps(alpha, beta).sum())
    mem_bound = gsum * max_row * max_wcol

    # fp8 scale factors from data maxima / bounds (powers of two, exact)
    # xg bound: |xg[i,t]| <= max_i ||x[:,i]||_2 (per batch) * max_t ||gt[:,t]||_2
    xcol_max = float(
        np.sqrt(
            (x_in * x_in).sum(axis=0).max()  # sum over t for each (b, i)
        )
    )
    gtcol_max = float(np.sqrt((gt_np.astype(np.float64) ** 2).sum(axis=0).max()))
    xg_bound = xcol_max * gtcol_max
    w_max = float(np.abs(W_in64).max())
    sx = _pow2_scale(224.0, xg_bound)
    sw = _pow2_scale(224.0, w_max)

    # --- mixed-precision error allowance (conservative, absolute) -------
    # bf16 stage-1 relative error ~<1%; fp8 e4m3 operand rounding <=2^-4
    # relative each plus subnormal-flush floors eps = 2^-9/scale.
    eps_x = 2.0**-9 / sx
    eps_w = 2.0**-9 / sw
    err = (
        0.15 * mem_bound
        + IN * (eps_x * w_max + eps_w * xg_bound + eps_x * eps_w)
    )
    safe = (mem_bound + err) < float(threshold.min()) - MARGIN
    if not safe:
        return _fallback(input_signal, weights, tau_mem, tau_syn, threshold)

    # batch-major rows: row (b*T + t) = input_signal[t, b, :IN]
    x_bm = np.ascontiguousarray(
        input_signal[:, :, :IN].transpose(1, 0, 2).reshape(B * T, IN)
    ).astype(np.float32, copy=False)
    W_in = np.ascontiguousarray(weights[:IN]).astype(np.float32, copy=False)

    try:
        spikes, _ = _run_device(
            x_bm, W_in, gt_np, threshold.astype(np.float32), sx, sw,
            trace=_trace,
        )
    except Exception:  # device unusable: still return a correct result
        return _fallback(input_signal, weights, tau_mem, tau_syn, threshold)
    if spikes.any():
        # bound said sub-threshold yet device saw spikes: distrust, recompute
        return _fallback(input_signal, weights, tau_mem, tau_syn, threshold)
    return spikes
